# revision 47
# baseline (speedup 1.0000x reference)
"""GCCF (gnn message passing) Bass kernel for 8 trn2 NeuronCores.

Model (reference.py):
  3 layers of bipartite graph propagation:
    u_l = LReLU((user_adj @ m_{l-1} + u_{l-1}) @ Wu[l].T + 2*bu[l])
    m_l = LReLU((movie_adj @ u_{l-1} + m_{l-1}) @ Wm[l].T + 2*bm[l])
  then 100k (uid, mid) pair interactions:
    out[b] = sum_l (u_l[uid] * m_l[mid]) . wo_l + bo

Distribution (8 cores):
  - adjacency rows sharded: core c owns users [2000c, 2000c+2000) and
    movies [1000c, 1000c+1000); each core computes its slice of u_l/m_l
    against the full (all-gathered) opposite-side embedding.
  - adjacency is transposed + scaled (x 2^17) + cast to fp8 e4m3 on the
    HOST, so every layer just streams [128, 4, W] adjacency tiles and
    matmuls them against fp8 embedding stationaries; the 2^17 scale is
    compensated in the epilogue activation's `scale` and by scaling the
    residual embedding once per layer-side.
  - interaction pairs are bucketed by uid owner on the host; the pair
    term is ONE dot product over 256 dims per pair:
      out[p] = U[uid_p] . M[mid_p],
      U[u] = [u0*wo0 | u1*wo1 | u2*wo2 | u3*wo3]   (local users, bf16)
      M[m] = [m0 | m1 | m2 | m3]                   (all movies, bf16)
    so each pair needs exactly TWO 512B dma_gather rows.  dma_gather
    descriptor generation is the scarce resource (~8.4ns/idx serial on
    GpSimd), so gathers are minimized (2 per pair) and ordered so the
    m-row gathers overlap the layer-3 u-side matmul stream; only the
    u-row gathers trail the compute.

Precision: adjacency fp8 e4m3, stationary embeddings fp8, residual path
fp32, interaction tables bf16.
"""
import os
import sys
import threading

sys.path.insert(0, "/opt/trn_rl_repo")

import numpy as np
import ml_dtypes

import concourse.bacc as bacc
import concourse.mybir as mybir
import concourse.tile as tile
from concourse.bass_utils import run_bass_kernel_spmd
from concourse.instruction_name_ordered_set import InstructionNameOrderedSet
from concourse.masks import make_identity

dt = mybir.dt
F32, BF16, I16, F8 = dt.float32, dt.bfloat16, dt.int16, dt.float8e4
F32R = dt.float32r
NPF8 = ml_dtypes.float8_e4m3
NPBF16 = ml_dtypes.bfloat16
ALU = mybir.AluOpType
AXIS = mybir.AxisListType
ACTF = mybir.ActivationFunctionType
DROW = mybir.MatmulPerfMode.DoubleRow

NCORES = 8
NU, NM, E, L, B = 16000, 8000, 64, 3, 100000
UPC, MPC = NU // NCORES, NM // NCORES        # rows per core: 2000 users, 1000 movies
UP, MP = 2048, 1024                          # padded to multiples of 512
MP8 = 8064                                   # movie table rows (63*128)
UKT, MKT = 125, 63                           # k-tiles: 125 (users), 62.5->63 (movies)
NUCH, NMCH = UP // 512, MP // 512            # output psum chunks per side (4, 2)
ASCALE = float(2 ** 17)                      # adjacency fp8 scale (e4m3: [0, 131])
CHUNK = 1024                                 # pairs per dma_gather (2048 wedges DGE)
SC = CHUNK // 128                            # result slots per chunk (8)
NW = CHUNK // 16                             # idx words per chunk
TE = 4 * E                                   # gather-table row width (256)


def _emit(nc, tc, io, nch):
    ctxs = []

    def pool(*a, **kw):
        p = tc.tile_pool(*a, **kw)
        ctxs.append(p)
        return p.__enter__()

    const = pool(name="const", bufs=1)
    ident_bf = const.tile([128, 128], BF16)
    make_identity(nc, ident_bf)
    ident_f32 = const.tile([128, 128], F32)
    make_identity(nc, ident_f32)

    # small constants: Wu^T/Wm^T per layer, biases, wo scales
    wut_sb, wmt_sb, bu2_sb, bm2_sb, wo4_sb = [], [], [], [], []
    for l in range(L):
        w = const.tile([64, 64], F32R, tag=f"wut{l}")
        nc.sync.dma_start(out=w[:], in_=io["wut"].ap()[l])
        wut_sb.append(w)
        w = const.tile([64, 64], F32R, tag=f"wmt{l}")
        nc.sync.dma_start(out=w[:], in_=io["wmt"].ap()[l])
        wmt_sb.append(w)
        bb = const.tile([64, 1], F32, tag=f"bu2{l}")
        nc.sync.dma_start(out=bb[:], in_=io["bu2"].ap()[l])
        bu2_sb.append(bb)
        bb = const.tile([64, 1], F32, tag=f"bm2{l}")
        nc.sync.dma_start(out=bb[:], in_=io["bm2"].ap()[l])
        bm2_sb.append(bb)
    for l in range(4):
        w = const.tile([64, 1], F32, tag=f"wo{l}")
        nc.sync.dma_start(out=w[:], in_=io["wo4"].ap()[l])
        wo4_sb.append(w)

    res_sb = const.tile([128, nch * SC], F32)
    const_objs = (ident_bf, ident_f32, wut_sb, wmt_sb, bu2_sb, bm2_sb, wo4_sb, res_sb)

    # ---- DRAM scratch (shared across repeat iterations) ----------------
    utab_d = nc.dram_tensor("utab_d", [UP, TE], BF16, kind="Internal")
    mtab_d = nc.dram_tensor("mtab_d", [MP8, TE], BF16, kind="Internal")
    # Alias of mtab for the LAST column write: the gather preps are emitted
    # before that write, and a tracked WAR (writer after a prepared read)
    # deadlocks — writer waits the prepped DMA, which waits the trigger,
    # which waits the writer.  The alias hides the WAR from Tile; a manual
    # sync dep on the trigger restores the real ordering.
    mtab3_d = nc.dram_tensor("mtab3_d", [MP8, TE], BF16, kind="Internal")
    nc.lookup_mls(mtab3_d).memorylocations[0].addr = \
        nc.lookup_mls(mtab_d).memorylocations[0].addr
    utab3_d = nc.dram_tensor("utab3_d", [UP, TE], BF16, kind="Internal")
    nc.lookup_mls(utab3_d).memorylocations[0].addr = \
        nc.lookup_mls(utab_d).memorylocations[0].addr
    agu_in = [
        nc.dram_tensor(f"agu_in{l}", [UPC, 64], F8, kind="Internal")
        for l in range(1, 3)
    ]
    agu_out = [
        nc.dram_tensor(f"agu_out{l}", [NU, 64], F8, kind="Internal",
                       addr_space="Shared")
        for l in range(1, 3)
    ]
    agm_in = [
        nc.dram_tensor(f"agm_in{l}", [MPC, 64], BF16, kind="Internal")
        for l in range(1, 4)
    ]
    agm_out = [
        nc.dram_tensor(f"agm_out{l}", [NM, 64], BF16, kind="Internal",
                       addr_space="Shared")
        for l in range(1, 4)
    ]

    _kphase = os.environ.get("KPHASE", "full")
    _nlayers = {"setup": 0, "l1": 1, "l2": 2, "l3": 3, "nogather": 3}.get(_kphase, L)
    _gather = _kphase in ("full", "gather")
    _krep = int(os.environ.get("KREPEAT", "1"))
    for _it in range(_krep):
        _emit_iter(nc, tc, io, const_objs, _nlayers, _gather, nch,
                   utab_d, mtab_d, mtab3_d, utab3_d, agu_in, agu_out,
                   agm_in, agm_out)

    for p in reversed(ctxs):
        p.__exit__(None, None, None)


def _emit_iter(nc, tc, io, const_objs, _nlayers, _gather, nch,
               utab_d, mtab_d, mtab3_d, utab3_d, agu_in, agu_out,
               agm_in, agm_out):
    (ident_bf, ident_f32, wut_sb, wmt_sb, bu2_sb, bm2_sb, wo4_sb, res_sb) = const_objs
    ctxs = []

    def pool(*a, **kw):
        p = tc.tile_pool(*a, **kw)
        ctxs.append(p)
        return p.__enter__()

    # ---- pools ---------------------------------------------------------
    autp = pool(name="auT", bufs=2)
    amtp = pool(name="amT", bufs=3)
    ustatp = pool(name="ustat", bufs=2)
    mstatp = pool(name="mstat", bufs=2)
    stgp = pool(name="stg", bufs=2)
    utp = pool(name="uT", bufs=2)
    mtp = pool(name="mT", bufs=2)
    uhtp = pool(name="uhatT", bufs=1)
    ubfp = pool(name="ubf", bufs=1)
    xp = pool(name="x", bufs=2)
    s64p = pool(name="s64", bufs=3)
    gmp = pool(name="gm", bufs=1)
    gup = pool(name="gu", bufs=1)
    gi = pool(name="gi", bufs=1)
    accp = pool(name="acc", bufs=4, space="PSUM")
    tpp = pool(name="tp", bufs=2, space="PSUM")
    ps2p = pool(name="ps2", bufs=2, space="PSUM")

    # ---- helper: stationary builders -----------------------------------
    def load_stat_u(src_ap):
        """DRAM [16000, 64] fp8 -> SBUF [128, 125, 64] fp8.

        GPSIMD queue: the load waits on the u-AG, and on the sync queue it
        would head-of-line-block the next side's adjacency stream loads."""
        st = ustatp.tile([128, UKT, 64], F8, tag="stat")
        nc.gpsimd.dma_start(
            out=st[:], in_=src_ap.rearrange("(a p) e -> p a e", p=128)
        )
        return st

    def cast_stat_m(src_ap):
        """DRAM [8000, 64] bf16 -> SBUF [128, 63, 64] fp8 (pad zeroed).

        Staging DMAs ride the GPSIMD queue: they wait on the AllGather, and
        on the sync queue they would head-of-line-block the next side's
        adjacency stream loads behind that wait."""
        st = mstatp.tile([128, MKT, 64], F8, tag="stat")
        nc.vector.memset(st[64:, 62, :], 0.0)
        src3 = src_ap[:7936].rearrange("(a p) e -> p a e", p=128)
        CHK = 16
        for s in range(0, 62, CHK):
            w = min(CHK, 62 - s)
            stg = stgp.tile([128, CHK, 64], BF16, tag="stg")
            nc.gpsimd.dma_start(out=stg[:, :w, :], in_=src3[:, s : s + w, :])
            nc.gpsimd.tensor_copy(out=st[:, s : s + w, :], in_=stg[:, :w, :])
        stg = stgp.tile([128, CHK, 64], BF16, tag="stg")
        nc.gpsimd.dma_start(out=stg[:64, 0, :], in_=src_ap[7936:])
        nc.gpsimd.tensor_copy(out=st[:64, 62, :], in_=stg[:64, 0, :])
        return st

    # ---- big matmul: stream fp8 adjacency tiles -------------------------
    def _pairs(mt, stat, psums, b, npair, first):
        for j in range(npair):
            k2 = b * 4 + j * 2
            for n, ps in enumerate(psums):
                nc.tensor.matmul(
                    ps[:],
                    stat[:, k2 : k2 + 2, :],
                    mt[:, j * 2 : j * 2 + 2, n * 512 : (n + 1) * 512],
                    start=(first and j == 0 and n is not None and k2 == 0),
                    stop=False,
                    perf_mode=DROW,
                )

    def stream_u(stat, psums):
        """auT [8000, 2048] fp8: 15x4-tile batches + 2 tiles + 64-row tail."""
        for b in range(15):
            mt = autp.tile([128, 4, UP], F8, tag="auT")
            nc.sync.dma_start(
                out=mt[:],
                in_=io["auT"].ap()[b * 512 : (b + 1) * 512].rearrange(
                    "(a p) c -> p a c", p=128
                ),
            )
            _pairs(mt, stat, psums, b, 2, b == 0)
        mt = autp.tile([128, 4, UP], F8, tag="auT")
        nc.sync.dma_start(
            out=mt[:, :2, :],
            in_=io["auT"].ap()[7680:7936].rearrange("(a p) c -> p a c", p=128),
        )
        nc.sync.dma_start(out=mt[:64, 2, :], in_=io["auT"].ap()[7936:])
        _pairs(mt, stat, psums, 15, 1, False)
        for n, ps in enumerate(psums):
            nc.tensor.matmul(
                ps[:],
                stat[0:64, 62, :],
                mt[0:64, 2, n * 512 : (n + 1) * 512],
                start=False,
                stop=True,
            )

    def stream_m(stat, psums):
        """amT [16000, 1024] fp8: 31x4-tile batches + one 128-row tail."""
        for b in range(31):
            mt = amtp.tile([128, 4, MP], F8, tag="amT")
            nc.sync.dma_start(
                out=mt[:],
                in_=io["amT"].ap()[b * 512 : (b + 1) * 512].rearrange(
                    "(a p) c -> p a c", p=128
                ),
            )
            _pairs(mt, stat, psums, b, 2, b == 0)
        mt = amtp.tile([128, 4, MP], F8, tag="amT")
        nc.sync.dma_start(out=mt[:, 0, :], in_=io["amT"].ap()[15872:])
        for n, ps in enumerate(psums):
            nc.tensor.matmul(
                ps[:],
                stat[:, 124, :],
                mt[:, 0, n * 512 : (n + 1) * 512],
                start=False,
                stop=True,
            )

    # ---- epilogue: x = psum + prevT*S; x @ W^T; LReLU(x/S) --------------
    def epilogue(psums, prevT_s, w_sb, b_sb, outp, width, tag, chunk_cb=None):
        curT = outp.tile([64, width], F32, tag=tag)
        for n, ps in enumerate(psums):
            x = xp.tile([64, 512], F32R, tag="x")
            nc.vector.tensor_tensor(
                x[:], ps[:], prevT_s[:, n * 512 : (n + 1) * 512], ALU.add
            )
            ps2 = ps2p.tile([64, 512], F32, tag="ps2")
            nc.tensor.matmul(ps2[:], w_sb[:], x[:], start=True, stop=True)
            nc.scalar.activation(
                curT[:, n * 512 : (n + 1) * 512],
                ps2[:],
                ACTF.Lrelu,
                bias=b_sb[:],
                scale=1.0 / ASCALE,
                alpha=0.01,
            )
            if chunk_cb is not None:
                chunk_cb(curT, n)
        return curT

    def transpose_out(srcT, cols, dst_ap, ident, sdt, odt=None):
        """[64, >=cols] srcT -> natural [cols, 64] written to dst_ap rows.

        odt: optional output dtype; the DVE copy after the PE transpose
        converts (e.g. f32 -> bf16 for the AG payload)."""
        odt = odt or sdt
        dmas = []
        for i in range(0, cols, 128):
            cw = min(128, cols - i)
            tp = tpp.tile([128, 128], sdt, tag="tp")
            nc.tensor.transpose(tp[:cw, :64], srcT[:, i : i + cw], ident[0:64, 0:64])
            sbt = s64p.tile([128, 64], odt, tag="s64" + str(odt))
            nc.vector.tensor_copy(out=sbt[:cw, :], in_=tp[:cw, :64])
            dmas.append(
                nc.sync.dma_start(out=dst_ap[i : i + cw, :], in_=sbt[:cw, :])
            )
        return dmas

    def allgather(in_t, out_t):
        if os.environ.get("KSIM") or os.environ.get("KNOCC"):
            # timing-only variant: emulate AG with local DMAs (wrong results)
            n = in_t.ap().shape[0]
            for r in range(NCORES):
                nc.sync.dma_start(
                    out=out_t.ap()[r * n : (r + 1) * n, :], in_=in_t.ap()
                )
            return
        nc.gpsimd.collective_compute(
            "AllGather",
            ALU.bypass,
            replica_groups=[list(range(NCORES))],
            ins=[in_t.ap().opt()],
            outs=[out_t.ap().opt()],
        )

    # ---- interaction gathers --------------------------------------------
    uidx_sb = gi.tile([128, nch * NW], I16)
    midx_sb = gi.tile([128, nch * NW], I16)

    gsem_m = [nc.alloc_semaphore(f"sw_gm{c}") for c in range(nch)]
    gsem_u = [nc.alloc_semaphore(f"sw_gu{c}") for c in range(nch)]
    grelay = [nc.alloc_semaphore(f"gdr{c}") for c in range(nch)]

    def _dgather(out_tile, tab_ap, idx_sb, ch, prep=False, q=1):
        # prepped gathers ride SWDGE queues 1/2: their untriggered
        # descriptors would otherwise block later self-triggered DMAs in
        # queue 0's FIFO.  Per-chunk completion sems let the consume of
        # chunk ch start as soon as ITS two rows-batches landed.
        nc.gpsimd.dma_gather(
            out_ap=out_tile[:],
            in_ap=tab_ap,
            idxs_ap=idx_sb[:, ch * NW : (ch + 1) * NW],
            num_idxs=CHUNK,
            num_idxs_reg=CHUNK,
            elem_size=TE,
            prepare_only=prep,
            sem=(gsem_m[ch] if q == 1 else gsem_u[ch]) if prep else None,
            queue_num=q if prep else 0,
        )

    def write_u_col(uT_nat, l, dst_t=None):
        """U table column block l (l=1..3): wo_l-scaled, transposed."""
        dst_t = dst_t or utab_d
        uhatT = uhtp.tile([64, UP], BF16, tag="uhatT")
        nc.vector.tensor_scalar_mul(uhatT[:], uT_nat[:], wo4_sb[l][:])
        return transpose_out(
            uhatT, UP, dst_t.ap()[:, l * 64 : (l + 1) * 64], ident_bf, BF16
        )

    def write_m_col(src_ap, l, dst_t=None, eng=None):
        """agm_out [8000, 64] bf16 -> mtab column block l, direct DRAM->DRAM.

        Default GPSIMD queue (the read waits on the AG; on sync it would
        head-of-line-block the adjacency stream).  The exposed l3 write
        passes eng=nc.scalar (fast HWDGE issue, free queue by then).
        Returns the DMA instructions (for manual trigger deps)."""
        dst_t = dst_t or mtab_d
        eng = eng or nc.gpsimd
        return [eng.dma_start(
            out=dst_t.ap()[:8000, l * 64 : (l + 1) * 64], in_=src_ap
        )]

    # ---- setup (order = sync-engine queue order: L1-m deps first) -------
    u_stats = {}
    m_stats = {}
    st = ustatp.tile([128, UKT, 64], F8, tag="stat")
    # split load: first 16 k-tiles land fast so L1-m matmuls start early
    nc.sync.dma_start(out=st[:, :16, :], in_=io["ustat0"].ap()[:, :16, :])
    nc.sync.dma_start(out=st[:, 16:, :], in_=io["ustat0"].ap()[:, 16:, :])
    u_stats[0] = st
    mT = mtp.tile([64, MP], F32, tag="mT")
    nc.sync.dma_start(out=mT[:], in_=io["meT"].ap())
    st = mstatp.tile([128, MKT, 64], F8, tag="stat")
    nc.sync.dma_start(out=st[:], in_=io["mstat0"].ap())
    m_stats[0] = st
    uT = utp.tile([64, UP], F32, tag="uT")          # scaled by ASCALE (host)
    nc.sync.dma_start(out=uT[:], in_=io["ueT"].ap())
    # gather-table init: column 0 host-prefilled, cols 1-3 zeroed (scalar
    # queue, off the adjacency-stream critical path)
    nc.scalar.dma_start(out=utab_d.ap(), in_=io["u0tab"].ap())
    nc.scalar.dma_start(out=mtab_d.ap(), in_=io["m0tab"].ap())
    uT_nat, mT_nat = None, None                      # natural-scale outputs
    mgs = []                                         # gathered m-row tiles
    ugs = []                                         # gathered u-row tiles
    ucol_dmas = []                                   # utab col-2/3 write DMAs
    mcol_dmas = []                                   # mtab col-2/3 write DMAs
    mtrig = [None]                                   # m-gather trigger inst

    # ---- 3 layers --------------------------------------------------------
    for l in range(_nlayers):
        # side order: L1=[m,u], L2=[u,m], L3=[m,u]  (zero AG stalls)
        m_first = l != 1

        def do_m(l=l):
            nonlocal mT, mT_nat
            if l > 0:
                nc.vector.tensor_scalar_mul(mT[:], mT[:], ASCALE)
            mTs = mT
            psums = [accp.tile([64, 512], F32, tag="acc", name=f"psm{i}") for i in range(NMCH)]
            stream_m(u_stats[l], psums)
            mT_nat = epilogue(psums, mTs, wmt_sb[l], bm2_sb[l], mtp, MP, "mT")
            mT = mT_nat
            transpose_out(mT_nat, MPC, agm_in[l].ap(), ident_f32, F32, odt=BF16)
            allgather(agm_in[l], agm_out[l])
            if l < 2:
                m_stats[l + 1] = cast_stat_m(agm_out[l].ap())
            if l == 0:
                write_m_col(agm_out[l].ap(), 1)
            elif l == 1:
                # col 2 lands after the prep emission: alias + manual dep
                mcol_dmas.extend(write_m_col(agm_out[l].ap(), 2, mtab3_d))
            # l == 2: the col-3 write is sequenced by the caller (it goes
            # through the mtab alias, after the gather preps)

        def do_u(l=l):
            nonlocal uT, uT_nat
            if l > 0:
                nc.vector.tensor_scalar_mul(uT[:], uT[:], ASCALE)
            uTs = uT
            psums = [accp.tile([64, 512], F32, tag="acc", name=f"psu{i}") for i in range(NUCH)]
            stream_u(m_stats[l], psums)
            if l == 2:
                # fuse the utab col-3 write into the epilogue: each 512-col
                # activation chunk is immediately wo-scaled, transposed and
                # written, so the u-gather trigger fires ~30us earlier.
                uhatT = uhtp.tile([64, UP], BF16, tag="uhatT")
                dst_ap = utab3_d.ap()[:, 3 * 64 : 4 * 64]

                def u3_cb(curT, n):
                    sl = slice(n * 512, (n + 1) * 512)
                    nc.vector.tensor_scalar_mul(
                        uhatT[:, sl], curT[:, sl], wo4_sb[3][:]
                    )
                    ucol_dmas.extend(transpose_out(
                        uhatT[:, sl], 512, dst_ap[n * 512 : (n + 1) * 512, :],
                        ident_bf, BF16,
                    ))
            else:
                u3_cb = None
            uT_nat = epilogue(psums, uTs, wut_sb[l], bu2_sb[l], utp, UP, "uT",
                              chunk_cb=u3_cb)
            uT = uT_nat
            # AG chain first: the next layer's stationary is the critical path
            if l < 2:
                ubf = ubfp.tile([64, UP], BF16, tag="ubf")
                nc.vector.tensor_copy(out=ubf[:], in_=uT_nat[:])
                transpose_out(ubf, UPC, agu_in[l].ap(), ident_bf, BF16, odt=F8)
                allgather(agu_in[l], agu_out[l])
                u_stats[l + 1] = load_stat_u(agu_out[l].ap())
            if l == 0:
                write_u_col(uT_nat, 1)
            elif l == 1:
                # col 2 lands after the prep emission: alias + manual dep
                ucol_dmas.extend(write_u_col(uT_nat, 2, utab3_d))
            # l == 2: col 3 was written by the fused epilogue callback

        if m_first:
            do_m()
            if _gather and l == 2:
                # column 3 through the alias (see mtab3_d comment), then the
                # trigger with manual deps on the col-2/3 writes.
                mcol_dmas.extend(write_m_col(agm_out[2].ap(), 3, mtab3_d, eng=nc.scalar))
                mtrig[0] = nc.gpsimd.trigger_dma(count=None, queue_num=1)
                deps = InstructionNameOrderedSet()
                for dma in mcol_dmas:
                    deps.add(dma.ins.name)
                mtrig[0].ins.add_sync_dependencies_from(deps)
            do_u()
        else:
            do_u()
            do_m()

        if l == 0:
            # small loads, deferred off the startup critical path
            nc.sync.dma_start(out=uidx_sb[:], in_=io["uidx"].ap())
            nc.sync.dma_start(out=midx_sb[:], in_=io["midx"].ap())
            if _gather and _nlayers > 2:
                # ALL gather preps, emitted at end of L1: descriptor
                # generation (2 x nch x 8.6us, serial on GpSimd Q7) fills
                # GpSimd idle time across L2/L3.  Table columns written
                # after this point go through the alias tensors; the
                # triggers carry manual deps on those writes.
                for ch in range(nch):
                    mg = gmp.tile([128, SC, TE], BF16, tag=f"mg{ch}")
                    _dgather(mg, mtab_d.ap(), midx_sb, ch, prep=True, q=1)
                    mgs.append(mg)
                for ch in range(nch):
                    ug = gup.tile([128, SC, TE], BF16, tag=f"ug{ch}")
                    _dgather(ug, utab_d.ap(), uidx_sb, ch, prep=True, q=2)
                    ugs.append(ug)
        if _gather and l == 2:
            # fire the prepped u-row gathers once utab cols 2,3 have landed
            utrig = nc.gpsimd.trigger_dma(count=None, queue_num=2)
            udeps = InstructionNameOrderedSet()
            for dma in ucol_dmas:
                udeps.add(dma.ins.name)
            utrig.ins.add_sync_dependencies_from(udeps)
            # The per-chunk sems (exact, not cumulative) gate each chunk's
            # consume via a relay that requires BOTH its gathers: two
            # GpSimd waits bump grelay[ch]; the consume waits grelay[ch]>=2.
            # no_sync deps pin the waits after the triggers in the GpSimd
            # stream (a hoisted wait before the trigger would deadlock).
            tdeps = InstructionNameOrderedSet()
            tdeps.add(mtrig[0].ins.name)
            tdeps.add(utrig.ins.name)
            for ch in range(nch):
                w1 = nc.gpsimd.wait_ge(gsem_m[ch], 16).then_inc(grelay[ch], 1)
                w1.ins.add_nosync_dependencies_from(tdeps)
                w2 = nc.gpsimd.wait_ge(gsem_u[ch], 16).then_inc(grelay[ch], 1)
                w2.ins.add_nosync_dependencies_from(tdeps)
            # consume: per-chunk product (in place, into the m tile)+reduce
            for ch in range(nch):
                mult = nc.vector.tensor_tensor(
                    mgs[ch][:], ugs[ch][:], mgs[ch][:], ALU.mult
                )
                mult._wait_ge(grelay[ch], 2)
                nc.vector.tensor_reduce(
                    res_sb[:, ch * SC : (ch + 1) * SC], mgs[ch][:],
                    axis=AXIS.X, op=ALU.add,
                )

    if not _gather:
        nc.any.memset(res_sb[:], 0.0)

    nc.sync.dma_start(out=io["res"].ap(), in_=res_sb[:])

    for p in reversed(ctxs):
        p.__exit__(None, None, None)


def _build(nch):
    ndev = 1 if os.environ.get("KSIM") else NCORES
    nc = bacc.Bacc("TRN2", num_devices=ndev, debug=False, num_swdge_queues=3)
    io = {}
    io["auT"] = nc.dram_tensor("auT", [NM, UP], F8, kind="ExternalInput")
    io["amT"] = nc.dram_tensor("amT", [NU, MP], F8, kind="ExternalInput")
    io["ustat0"] = nc.dram_tensor("ustat0", [128, UKT, E], F8, kind="ExternalInput")
    io["mstat0"] = nc.dram_tensor("mstat0", [128, MKT, E], F8, kind="ExternalInput")
    io["ueT"] = nc.dram_tensor("ueT", [E, UP], F32, kind="ExternalInput")
    io["meT"] = nc.dram_tensor("meT", [E, MP], F32, kind="ExternalInput")
    io["u0tab"] = nc.dram_tensor("u0tab", [UP, TE], BF16, kind="ExternalInput")
    io["m0tab"] = nc.dram_tensor("m0tab", [MP8, TE], BF16, kind="ExternalInput")
    io["wut"] = nc.dram_tensor("wut", [L, E, E], F32R, kind="ExternalInput")
    io["wmt"] = nc.dram_tensor("wmt", [L, E, E], F32R, kind="ExternalInput")
    io["bu2"] = nc.dram_tensor("bu2", [L, E, 1], F32, kind="ExternalInput")
    io["bm2"] = nc.dram_tensor("bm2", [L, E, 1], F32, kind="ExternalInput")
    io["wo4"] = nc.dram_tensor("wo4", [4, E, 1], F32, kind="ExternalInput")
    io["uidx"] = nc.dram_tensor("uidx", [128, nch * NW], I16, kind="ExternalInput")
    io["midx"] = nc.dram_tensor("midx", [128, nch * NW], I16, kind="ExternalInput")
    io["res"] = nc.dram_tensor("res", [128, nch * SC], F32, kind="ExternalOutput")

    with tile.TileContext(nc) as tc:
        _emit(nc, tc, io, nch)
    nc.compile()
    return nc


_cache = threading.local()


def _get_nc(nch):
    key = ("nc", nch)
    nc = getattr(_cache, "store", {}).get(key)
    if nc is None:
        nc = _build(nch)
        if not hasattr(_cache, "store"):
            _cache.store = {}
        _cache.store[key] = nc
    return nc


def _wrap_idx(arr, nch):
    """[CAP] int16 -> [128, nch*NW] wrapped layout for dma_gather."""
    w = arr.reshape(nch, NW, 16).transpose(2, 0, 1)   # [16, nch, NW]
    w = np.tile(w, (8, 1, 1)).reshape(128, nch * NW)
    return np.ascontiguousarray(w)


def _prep_in_maps(user_adj, movie_adj, user_emb, movie_emb, Wu, bu, Wm, bm,
                  Wo, bo, user_id, movie_id):
    user_adj = np.asarray(user_adj, np.float32)
    movie_adj = np.asarray(movie_adj, np.float32)
    user_emb = np.asarray(user_emb, np.float32)
    movie_emb = np.asarray(movie_emb, np.float32)
    Wu, bu = np.asarray(Wu, np.float32), np.asarray(bu, np.float32)
    Wm, bm = np.asarray(Wm, np.float32), np.asarray(bm, np.float32)
    Wo, bo = np.asarray(Wo, np.float32), np.asarray(bo, np.float32)
    user_id = np.asarray(user_id, np.int32)
    movie_id = np.asarray(movie_id, np.int32)

    wo = Wo[0]                                            # [(L+1)*E]
    wut = np.ascontiguousarray(Wu.transpose(0, 2, 1))
    wmt = np.ascontiguousarray(Wm.transpose(0, 2, 1))
    bu2 = np.ascontiguousarray((2.0 * bu).reshape(L, E, 1))
    bm2 = np.ascontiguousarray((2.0 * bm).reshape(L, E, 1))
    wo4 = np.ascontiguousarray(wo.reshape(4, E, 1))

    # stationaries: full embeddings, fp8 e4m3, [128, kt, 64] (zero-padded)
    uu = user_emb.astype(NPF8)                            # 16000 = 125*128
    ustat0 = np.ascontiguousarray(uu.reshape(UKT, 128, E).transpose(1, 0, 2))
    mm = np.zeros((MKT * 128, E), NPF8)                   # pad 8000 -> 8064
    mm[:NM] = movie_emb.astype(NPF8)
    mstat0 = np.ascontiguousarray(mm.reshape(MKT, 128, E).transpose(1, 0, 2))

    # movie table init: column 0 = natural movie embedding, rest zero
    m0tab = np.zeros((MP8, TE), NPBF16)
    m0tab[:NM, :E] = movie_emb.astype(NPBF16)

    # bucket pairs by uid owner
    own = user_id // UPC
    order = np.argsort(own, kind="stable")
    counts = np.bincount(own, minlength=NCORES)
    nch = max(1, -(-int(counts.max()) // CHUNK))
    cap = nch * CHUNK
    starts = np.zeros(NCORES + 1, np.int64)
    np.cumsum(counts, out=starts[1:])

    in_maps = []
    metas = []
    for c in range(NCORES):
        idx_c = order[starts[c] : starts[c + 1]]
        n_c = len(idx_c)
        uid_re = np.zeros(cap, np.int16)
        mid_c = np.zeros(cap, np.int16)
        uid_re[:n_c] = (user_id[idx_c] - c * UPC).astype(np.int16)
        mid_c[:n_c] = movie_id[idx_c].astype(np.int16)

        auT = np.zeros((NM, UP), NPF8)
        auT[:, :UPC] = (
            user_adj[c * UPC : (c + 1) * UPC].T * np.float32(ASCALE)
        ).astype(NPF8)
        amT = np.zeros((NU, MP), NPF8)
        amT[:, :MPC] = (
            movie_adj[c * MPC : (c + 1) * MPC].T * np.float32(ASCALE)
        ).astype(NPF8)

        ue_sl = np.zeros((UP, E), np.float32)
        ue_sl[:UPC] = user_emb[c * UPC : (c + 1) * UPC]
        me_sl = np.zeros((MP, E), np.float32)
        me_sl[:MPC] = movie_emb[c * MPC : (c + 1) * MPC]

        # U table init: column 0 = u0 * wo0 (local users, bf16), rest zero
        u0tab = np.zeros((UP, TE), NPBF16)
        u0tab[:, :E] = (ue_sl * wo[:E][None, :]).astype(NPBF16)

        in_maps.append({
            "auT": auT,
            "amT": amT,
            "ustat0": ustat0,
            "mstat0": mstat0,
            "ueT": np.ascontiguousarray(ue_sl.T) * np.float32(ASCALE),
            "meT": np.ascontiguousarray(me_sl.T) * np.float32(ASCALE),
            "u0tab": u0tab,
            "m0tab": m0tab,
            "wut": wut,
            "wmt": wmt,
            "bu2": bu2,
            "bm2": bm2,
            "wo4": wo4,
            "uidx": _wrap_idx(uid_re, nch),
            "midx": _wrap_idx(mid_c, nch),
        })
        metas.append((idx_c, n_c))

    return in_maps, metas, float(bo[0]), nch


def _postprocess(results, metas, bo0, nch):
    cap = nch * CHUNK
    out = np.zeros(B, np.float32)
    for c in range(NCORES):
        idx_c, n_c = metas[c]
        r = results[c]["res"]                             # [128, nch*SC]
        vals = r.reshape(128, nch, SC).transpose(1, 2, 0).reshape(cap)
        out[idx_c] = vals[:n_c]
    return out + np.float32(bo0)


def kernel(user_adj, movie_adj, user_emb, movie_emb, Wu, bu, Wm, bm, Wo, bo,
           user_id, movie_id):
    in_maps, metas, bo0, nch = _prep_in_maps(
        user_adj, movie_adj, user_emb, movie_emb, Wu, bu, Wm, bm, Wo, bo,
        user_id, movie_id,
    )
    nc = _get_nc(nch)
    res = run_bass_kernel_spmd(nc, in_maps, core_ids=list(range(NCORES)))
    return _postprocess(res.results, metas, bo0, nch)


# revision 49
# speedup vs baseline: 1.0300x; 1.0300x over previous
"""GCCF (gnn message passing) Bass kernel for 8 trn2 NeuronCores.

Model (reference.py):
  3 layers of bipartite graph propagation:
    u_l = LReLU((user_adj @ m_{l-1} + u_{l-1}) @ Wu[l].T + 2*bu[l])
    m_l = LReLU((movie_adj @ u_{l-1} + m_{l-1}) @ Wm[l].T + 2*bm[l])
  then 100k (uid, mid) pair interactions:
    out[b] = sum_l (u_l[uid] * m_l[mid]) . wo_l + bo

Distribution (8 cores):
  - adjacency rows sharded: core c owns users [2000c, 2000c+2000) and
    movies [1000c, 1000c+1000); each core computes its slice of u_l/m_l
    against the full (all-gathered) opposite-side embedding.
  - adjacency is transposed + scaled (x 2^17) + cast to fp8 e4m3 on the
    HOST, so every layer just streams [128, 4, W] adjacency tiles and
    matmuls them against fp8 embedding stationaries; the 2^17 scale is
    compensated in the epilogue activation's `scale` and by scaling the
    residual embedding once per layer-side.
  - interaction pairs are bucketed by uid owner on the host; the pair
    term is ONE dot product over 256 dims per pair:
      out[p] = U[uid_p] . M[mid_p],
      U[u] = [u0*wo0 | u1*wo1 | u2*wo2 | u3*wo3]   (local users, bf16)
      M[m] = [m0 | m1 | m2 | m3]                   (all movies, bf16)
    so each pair needs exactly TWO 512B dma_gather rows.  dma_gather
    descriptor generation is the scarce resource (~8.4ns/idx serial on
    GpSimd), so gathers are minimized (2 per pair) and ordered so the
    m-row gathers overlap the layer-3 u-side matmul stream; only the
    u-row gathers trail the compute.

Precision: adjacency fp8 e4m3, stationary embeddings fp8, residual path
fp32, interaction tables bf16.
"""
import os
import sys
import threading

sys.path.insert(0, "/opt/trn_rl_repo")

import numpy as np
import ml_dtypes

import concourse.bacc as bacc
import concourse.mybir as mybir
import concourse.tile as tile
from concourse.bass_utils import run_bass_kernel_spmd
from concourse.instruction_name_ordered_set import InstructionNameOrderedSet
from concourse.masks import make_identity

dt = mybir.dt
F32, BF16, I16, F8 = dt.float32, dt.bfloat16, dt.int16, dt.float8e4
F32R = dt.float32r
NPF8 = ml_dtypes.float8_e4m3
NPBF16 = ml_dtypes.bfloat16
ALU = mybir.AluOpType
AXIS = mybir.AxisListType
ACTF = mybir.ActivationFunctionType
DROW = mybir.MatmulPerfMode.DoubleRow

NCORES = 8
NU, NM, E, L, B = 16000, 8000, 64, 3, 100000
UPC, MPC = NU // NCORES, NM // NCORES        # rows per core: 2000 users, 1000 movies
UP, MP = 2048, 1024                          # padded to multiples of 512
MP8 = 8064                                   # movie table rows (63*128)
UKT, MKT = 125, 63                           # k-tiles: 125 (users), 62.5->63 (movies)
NUCH, NMCH = UP // 512, MP // 512            # output psum chunks per side (4, 2)
ASCALE = float(2 ** 17)                      # adjacency fp8 scale (e4m3: [0, 131])
CHUNK = 1024                                 # pairs per dma_gather (2048 wedges DGE)
SC = CHUNK // 128                            # result slots per chunk (8)
NW = CHUNK // 16                             # idx words per chunk
TE = 4 * E                                   # gather-table row width (256)


def _emit(nc, tc, io, nch):
    ctxs = []

    def pool(*a, **kw):
        p = tc.tile_pool(*a, **kw)
        ctxs.append(p)
        return p.__enter__()

    const = pool(name="const", bufs=1)
    ident_bf = const.tile([128, 128], BF16)
    make_identity(nc, ident_bf)
    ident_f32 = const.tile([128, 128], F32)
    make_identity(nc, ident_f32)

    # small constants: Wu^T/Wm^T per layer, biases, wo scales
    wut_sb, wmt_sb, bu2_sb, bm2_sb, wo4_sb = [], [], [], [], []
    for l in range(L):
        w = const.tile([64, 64], F32R, tag=f"wut{l}")
        nc.sync.dma_start(out=w[:], in_=io["wut"].ap()[l])
        wut_sb.append(w)
        w = const.tile([64, 64], F32R, tag=f"wmt{l}")
        nc.sync.dma_start(out=w[:], in_=io["wmt"].ap()[l])
        wmt_sb.append(w)
        bb = const.tile([64, 1], F32, tag=f"bu2{l}")
        nc.sync.dma_start(out=bb[:], in_=io["bu2"].ap()[l])
        bu2_sb.append(bb)
        bb = const.tile([64, 1], F32, tag=f"bm2{l}")
        nc.sync.dma_start(out=bb[:], in_=io["bm2"].ap()[l])
        bm2_sb.append(bb)
    for l in range(4):
        w = const.tile([64, 1], F32, tag=f"wo{l}")
        nc.sync.dma_start(out=w[:], in_=io["wo4"].ap()[l])
        wo4_sb.append(w)

    res_sb = const.tile([128, nch * SC], F32)
    const_objs = (ident_bf, ident_f32, wut_sb, wmt_sb, bu2_sb, bm2_sb, wo4_sb, res_sb)

    # ---- DRAM scratch (shared across repeat iterations) ----------------
    utab_d = nc.dram_tensor("utab_d", [UP, TE], BF16, kind="Internal")
    mtab_d = nc.dram_tensor("mtab_d", [MP8, TE], BF16, kind="Internal")
    # Alias of mtab for the LAST column write: the gather preps are emitted
    # before that write, and a tracked WAR (writer after a prepared read)
    # deadlocks — writer waits the prepped DMA, which waits the trigger,
    # which waits the writer.  The alias hides the WAR from Tile; a manual
    # sync dep on the trigger restores the real ordering.
    mtab3_d = nc.dram_tensor("mtab3_d", [MP8, TE], BF16, kind="Internal")
    nc.lookup_mls(mtab3_d).memorylocations[0].addr = \
        nc.lookup_mls(mtab_d).memorylocations[0].addr
    utab3_d = nc.dram_tensor("utab3_d", [UP, TE], BF16, kind="Internal")
    nc.lookup_mls(utab3_d).memorylocations[0].addr = \
        nc.lookup_mls(utab_d).memorylocations[0].addr
    agu_in = [
        nc.dram_tensor(f"agu_in{l}", [UPC, 64], F8, kind="Internal")
        for l in range(1, 3)
    ]
    agu_out = [
        nc.dram_tensor(f"agu_out{l}", [NU, 64], F8, kind="Internal",
                       addr_space="Shared")
        for l in range(1, 3)
    ]
    agm_in = [
        nc.dram_tensor(f"agm_in{l}", [MPC, 64], BF16, kind="Internal")
        for l in range(1, 4)
    ]
    agm_out = [
        nc.dram_tensor(f"agm_out{l}", [NM, 64], BF16, kind="Internal",
                       addr_space="Shared")
        for l in range(1, 4)
    ]

    _kphase = os.environ.get("KPHASE", "full")
    _nlayers = {"setup": 0, "l1": 1, "l2": 2, "l3": 3, "nogather": 3}.get(_kphase, L)
    _gather = _kphase in ("full", "gather")
    _krep = int(os.environ.get("KREPEAT", "1"))
    for _it in range(_krep):
        _emit_iter(nc, tc, io, const_objs, _nlayers, _gather, nch,
                   utab_d, mtab_d, mtab3_d, utab3_d, agu_in, agu_out,
                   agm_in, agm_out)

    for p in reversed(ctxs):
        p.__exit__(None, None, None)


def _emit_iter(nc, tc, io, const_objs, _nlayers, _gather, nch,
               utab_d, mtab_d, mtab3_d, utab3_d, agu_in, agu_out,
               agm_in, agm_out):
    (ident_bf, ident_f32, wut_sb, wmt_sb, bu2_sb, bm2_sb, wo4_sb, res_sb) = const_objs
    ctxs = []

    def pool(*a, **kw):
        p = tc.tile_pool(*a, **kw)
        ctxs.append(p)
        return p.__enter__()

    # ---- pools ---------------------------------------------------------
    autp = pool(name="auT", bufs=2)
    amtp = pool(name="amT", bufs=3)
    ustatp = pool(name="ustat", bufs=2)
    mstatp = pool(name="mstat", bufs=2)
    stgp = pool(name="stg", bufs=2)
    utp = pool(name="uT", bufs=2)
    mtp = pool(name="mT", bufs=2)
    uhtp = pool(name="uhatT", bufs=1)
    ubfp = pool(name="ubf", bufs=1)
    xp = pool(name="x", bufs=2)
    s64p = pool(name="s64", bufs=3)
    gmp = pool(name="gm", bufs=1)
    gup = pool(name="gu", bufs=1)
    gi = pool(name="gi", bufs=1)
    accp = pool(name="acc", bufs=4, space="PSUM")
    tpp = pool(name="tp", bufs=2, space="PSUM")
    ps2p = pool(name="ps2", bufs=2, space="PSUM")

    # ---- helper: stationary builders -----------------------------------
    def load_stat_u(src_ap):
        """DRAM [16000, 64] fp8 -> SBUF [128, 125, 64] fp8.

        VECTOR queue: the load waits on the u-AG — on the sync queue it
        would head-of-line-block the next side's adjacency stream loads,
        and on GpSimd it queues behind gather descriptor generation.
        Split so the consuming stream can start on the early k-tiles."""
        st = ustatp.tile([128, UKT, 64], F8, tag="stat")
        src3 = src_ap.rearrange("(a p) e -> p a e", p=128)
        nc.scalar.dma_start(out=st[:, :24, :], in_=src3[:, :24, :])
        nc.scalar.dma_start(out=st[:, 24:, :], in_=src3[:, 24:, :])
        return st

    def cast_stat_m(src_ap):
        """DRAM [8000, 64] bf16 -> SBUF [128, 63, 64] fp8 (pad zeroed).

        Staging DMAs ride the GPSIMD queue: they wait on the AllGather, and
        on the sync queue they would head-of-line-block the next side's
        adjacency stream loads behind that wait."""
        st = mstatp.tile([128, MKT, 64], F8, tag="stat")
        nc.vector.memset(st[64:, 62, :], 0.0)
        src3 = src_ap[:7936].rearrange("(a p) e -> p a e", p=128)
        CHK = 16
        for s in range(0, 62, CHK):
            w = min(CHK, 62 - s)
            stg = stgp.tile([128, CHK, 64], BF16, tag="stg")
            nc.scalar.dma_start(out=stg[:, :w, :], in_=src3[:, s : s + w, :])
            nc.vector.tensor_copy(out=st[:, s : s + w, :], in_=stg[:, :w, :])
        stg = stgp.tile([128, CHK, 64], BF16, tag="stg")
        nc.scalar.dma_start(out=stg[:64, 0, :], in_=src_ap[7936:])
        nc.vector.tensor_copy(out=st[:64, 62, :], in_=stg[:64, 0, :])
        return st

    # ---- big matmul: stream fp8 adjacency tiles -------------------------
    def _pairs(mt, stat, psums, b, npair, first):
        for j in range(npair):
            k2 = b * 4 + j * 2
            for n, ps in enumerate(psums):
                nc.tensor.matmul(
                    ps[:],
                    stat[:, k2 : k2 + 2, :],
                    mt[:, j * 2 : j * 2 + 2, n * 512 : (n + 1) * 512],
                    start=(first and j == 0 and n is not None and k2 == 0),
                    stop=False,
                    perf_mode=DROW,
                )

    def stream_u(stat, psums):
        """auT [8000, 2048] fp8: 15x4-tile batches + 2 tiles + 64-row tail."""
        for b in range(15):
            mt = autp.tile([128, 4, UP], F8, tag="auT")
            nc.sync.dma_start(
                out=mt[:],
                in_=io["auT"].ap()[b * 512 : (b + 1) * 512].rearrange(
                    "(a p) c -> p a c", p=128
                ),
            )
            _pairs(mt, stat, psums, b, 2, b == 0)
        mt = autp.tile([128, 4, UP], F8, tag="auT")
        nc.sync.dma_start(
            out=mt[:, :2, :],
            in_=io["auT"].ap()[7680:7936].rearrange("(a p) c -> p a c", p=128),
        )
        nc.sync.dma_start(out=mt[:64, 2, :], in_=io["auT"].ap()[7936:])
        _pairs(mt, stat, psums, 15, 1, False)
        for n, ps in enumerate(psums):
            nc.tensor.matmul(
                ps[:],
                stat[0:64, 62, :],
                mt[0:64, 2, n * 512 : (n + 1) * 512],
                start=False,
                stop=True,
            )

    def stream_m(stat, psums):
        """amT [16000, 1024] fp8: 31x4-tile batches + one 128-row tail."""
        for b in range(31):
            mt = amtp.tile([128, 4, MP], F8, tag="amT")
            nc.sync.dma_start(
                out=mt[:],
                in_=io["amT"].ap()[b * 512 : (b + 1) * 512].rearrange(
                    "(a p) c -> p a c", p=128
                ),
            )
            _pairs(mt, stat, psums, b, 2, b == 0)
        mt = amtp.tile([128, 4, MP], F8, tag="amT")
        nc.sync.dma_start(out=mt[:, 0, :], in_=io["amT"].ap()[15872:])
        for n, ps in enumerate(psums):
            nc.tensor.matmul(
                ps[:],
                stat[:, 124, :],
                mt[:, 0, n * 512 : (n + 1) * 512],
                start=False,
                stop=True,
            )

    # ---- epilogue: x = psum + prevT*S; x @ W^T; LReLU(x/S) --------------
    def epilogue(psums, prevT_s, w_sb, b_sb, outp, width, tag, chunk_cb=None):
        curT = outp.tile([64, width], F32, tag=tag)
        for n, ps in enumerate(psums):
            x = xp.tile([64, 512], F32R, tag="x")
            nc.vector.tensor_tensor(
                x[:], ps[:], prevT_s[:, n * 512 : (n + 1) * 512], ALU.add
            )
            ps2 = ps2p.tile([64, 512], F32, tag="ps2")
            nc.tensor.matmul(ps2[:], w_sb[:], x[:], start=True, stop=True)
            nc.scalar.activation(
                curT[:, n * 512 : (n + 1) * 512],
                ps2[:],
                ACTF.Lrelu,
                bias=b_sb[:],
                scale=1.0 / ASCALE,
                alpha=0.01,
            )
            if chunk_cb is not None:
                chunk_cb(curT, n)
        return curT

    def transpose_out(srcT, cols, dst_ap, ident, sdt, odt=None):
        """[64, >=cols] srcT -> natural [cols, 64] written to dst_ap rows.

        odt: optional output dtype; the DVE copy after the PE transpose
        converts (e.g. f32 -> bf16 for the AG payload)."""
        odt = odt or sdt
        dmas = []
        for i in range(0, cols, 128):
            cw = min(128, cols - i)
            tp = tpp.tile([128, 128], sdt, tag="tp")
            nc.tensor.transpose(tp[:cw, :64], srcT[:, i : i + cw], ident[0:64, 0:64])
            sbt = s64p.tile([128, 64], odt, tag="s64" + str(odt))
            nc.vector.tensor_copy(out=sbt[:cw, :], in_=tp[:cw, :64])
            dmas.append(
                nc.sync.dma_start(out=dst_ap[i : i + cw, :], in_=sbt[:cw, :])
            )
        return dmas

    def allgather(in_t, out_t):
        if os.environ.get("KSIM") or os.environ.get("KNOCC"):
            # timing-only variant: emulate AG with local DMAs (wrong results)
            n = in_t.ap().shape[0]
            for r in range(NCORES):
                nc.sync.dma_start(
                    out=out_t.ap()[r * n : (r + 1) * n, :], in_=in_t.ap()
                )
            return
        nc.gpsimd.collective_compute(
            "AllGather",
            ALU.bypass,
            replica_groups=[list(range(NCORES))],
            ins=[in_t.ap().opt()],
            outs=[out_t.ap().opt()],
        )

    # ---- interaction gathers --------------------------------------------
    uidx_sb = gi.tile([128, nch * NW], I16)
    midx_sb = gi.tile([128, nch * NW], I16)

    gsem_m = [nc.alloc_semaphore(f"sw_gm{c}") for c in range(nch)]
    gsem_u = [nc.alloc_semaphore(f"sw_gu{c}") for c in range(nch)]
    grelay = [nc.alloc_semaphore(f"gdr{c}") for c in range(nch)]

    def _dgather(out_tile, tab_ap, idx_sb, ch, prep=False, q=1):
        # prepped gathers ride SWDGE queues 1/2: their untriggered
        # descriptors would otherwise block later self-triggered DMAs in
        # queue 0's FIFO.  Per-chunk completion sems let the consume of
        # chunk ch start as soon as ITS two rows-batches landed.
        nc.gpsimd.dma_gather(
            out_ap=out_tile[:],
            in_ap=tab_ap,
            idxs_ap=idx_sb[:, ch * NW : (ch + 1) * NW],
            num_idxs=CHUNK,
            num_idxs_reg=CHUNK,
            elem_size=TE,
            prepare_only=prep,
            sem=(gsem_m[ch] if q == 1 else gsem_u[ch]) if prep else None,
            queue_num=q if prep else 0,
        )

    def write_u_col(uT_nat, l, dst_t=None):
        """U table column block l (l=1..3): wo_l-scaled, transposed."""
        dst_t = dst_t or utab_d
        uhatT = uhtp.tile([64, UP], BF16, tag="uhatT")
        nc.vector.tensor_scalar_mul(uhatT[:], uT_nat[:], wo4_sb[l][:])
        return transpose_out(
            uhatT, UP, dst_t.ap()[:, l * 64 : (l + 1) * 64], ident_bf, BF16
        )

    def write_m_col(src_ap, l, dst_t=None, eng=None):
        """agm_out [8000, 64] bf16 -> mtab column block l, direct DRAM->DRAM.

        Default GPSIMD queue (the read waits on the AG; on sync it would
        head-of-line-block the adjacency stream).  The exposed l3 write
        passes eng=nc.scalar (fast HWDGE issue, free queue by then).
        Returns the DMA instructions (for manual trigger deps)."""
        dst_t = dst_t or mtab_d
        eng = eng or nc.gpsimd
        return [eng.dma_start(
            out=dst_t.ap()[:8000, l * 64 : (l + 1) * 64], in_=src_ap
        )]

    # ---- setup (order = sync-engine queue order: L1-m deps first) -------
    u_stats = {}
    m_stats = {}
    st = ustatp.tile([128, UKT, 64], F8, tag="stat")
    # split load: first 16 k-tiles land fast so L1-m matmuls start early
    nc.sync.dma_start(out=st[:, :16, :], in_=io["ustat0"].ap()[:, :16, :])
    nc.sync.dma_start(out=st[:, 16:, :], in_=io["ustat0"].ap()[:, 16:, :])
    u_stats[0] = st
    mT = mtp.tile([64, MP], F32, tag="mT")
    nc.sync.dma_start(out=mT[:], in_=io["meT"].ap())
    st = mstatp.tile([128, MKT, 64], F8, tag="stat")
    nc.sync.dma_start(out=st[:], in_=io["mstat0"].ap())
    m_stats[0] = st
    uT = utp.tile([64, UP], F32, tag="uT")          # scaled by ASCALE (host)
    nc.sync.dma_start(out=uT[:], in_=io["ueT"].ap())
    # gather-table init: column 0 host-prefilled, cols 1-3 zeroed (scalar
    # queue, off the adjacency-stream critical path)
    nc.scalar.dma_start(out=utab_d.ap(), in_=io["u0tab"].ap())
    nc.scalar.dma_start(out=mtab_d.ap(), in_=io["m0tab"].ap())
    uT_nat, mT_nat = None, None                      # natural-scale outputs
    mgs = []                                         # gathered m-row tiles
    ugs = []                                         # gathered u-row tiles
    ucol_dmas = []                                   # utab col-2/3 write DMAs
    mcol_dmas = []                                   # mtab col-2/3 write DMAs
    mtrig = [None]                                   # m-gather trigger inst

    # ---- 3 layers --------------------------------------------------------
    for l in range(_nlayers):
        # side order: L1=[m,u], L2=[u,m], L3=[m,u]  (zero AG stalls)
        m_first = l != 1

        def do_m(l=l):
            nonlocal mT, mT_nat
            if l > 0:
                nc.vector.tensor_scalar_mul(mT[:], mT[:], ASCALE)
            mTs = mT
            psums = [accp.tile([64, 512], F32, tag="acc", name=f"psm{i}") for i in range(NMCH)]
            stream_m(u_stats[l], psums)
            mT_nat = epilogue(psums, mTs, wmt_sb[l], bm2_sb[l], mtp, MP, "mT")
            mT = mT_nat
            transpose_out(mT_nat, MPC, agm_in[l].ap(), ident_f32, F32, odt=BF16)
            allgather(agm_in[l], agm_out[l])
            # stat cast is emitted by the caller (its scalar staging DMAs
            # wait on the m-AG and would head-of-line-block the epilogue
            # activations that sit after this point in the scalar stream)
            if l == 0:
                write_m_col(agm_out[l].ap(), 1)
            elif l == 1:
                # col 2 lands after the prep emission: alias + manual dep
                mcol_dmas.extend(write_m_col(agm_out[l].ap(), 2, mtab3_d))
            # l == 2: the col-3 write is sequenced by the caller (it goes
            # through the mtab alias, after the gather preps)

        def do_u(l=l):
            nonlocal uT, uT_nat
            if l > 0:
                nc.vector.tensor_scalar_mul(uT[:], uT[:], ASCALE)
            uTs = uT
            psums = [accp.tile([64, 512], F32, tag="acc", name=f"psu{i}") for i in range(NUCH)]
            stream_u(m_stats[l], psums)
            if l == 2:
                # fuse the utab col-3 write into the epilogue: each 512-col
                # activation chunk is immediately wo-scaled, transposed and
                # written, so the u-gather trigger fires ~30us earlier.
                uhatT = uhtp.tile([64, UP], BF16, tag="uhatT")
                dst_ap = utab3_d.ap()[:, 3 * 64 : 4 * 64]

                def u3_cb(curT, n):
                    sl = slice(n * 512, (n + 1) * 512)
                    nc.vector.tensor_scalar_mul(
                        uhatT[:, sl], curT[:, sl], wo4_sb[3][:]
                    )
                    ucol_dmas.extend(transpose_out(
                        uhatT[:, sl], 512, dst_ap[n * 512 : (n + 1) * 512, :],
                        ident_bf, BF16,
                    ))
            else:
                u3_cb = None
            uT_nat = epilogue(psums, uTs, wut_sb[l], bu2_sb[l], utp, UP, "uT",
                              chunk_cb=u3_cb)
            uT = uT_nat
            # AG chain first: the next layer's stationary is the critical path
            if l < 2:
                ubf = ubfp.tile([64, UP], BF16, tag="ubf")
                nc.vector.tensor_copy(out=ubf[:], in_=uT_nat[:])
                transpose_out(ubf, UPC, agu_in[l].ap(), ident_bf, BF16, odt=F8)
                allgather(agu_in[l], agu_out[l])
                u_stats[l + 1] = load_stat_u(agu_out[l].ap())
            if l == 0:
                write_u_col(uT_nat, 1)
            elif l == 1:
                # col 2 lands after the prep emission: alias + manual dep
                ucol_dmas.extend(write_u_col(uT_nat, 2, utab3_d))
            # l == 2: col 3 was written by the fused epilogue callback

        if m_first:
            do_m()
            if l == 0:
                do_u()
                # cast after L1-u's activations are emitted (scalar HOL)
                m_stats[1] = cast_stat_m(agm_out[0].ap())
            if _gather and l == 2:
                # column 3 through the alias (see mtab3_d comment), then the
                # trigger with manual deps on the col-2/3 writes.
                mcol_dmas.extend(write_m_col(agm_out[2].ap(), 3, mtab3_d))
                mtrig[0] = nc.gpsimd.trigger_dma(count=None, queue_num=1)
                deps = InstructionNameOrderedSet()
                for dma in mcol_dmas:
                    deps.add(dma.ins.name)
                mtrig[0].ins.add_sync_dependencies_from(deps)
            if l == 2:
                do_u()
        else:
            do_u()
            do_m()
            # cast after L2-m's activations are emitted (scalar HOL);
            # ready well before the L3-u stream needs it
            m_stats[2] = cast_stat_m(agm_out[1].ap())

        if l == 0:
            # small loads, deferred off the startup critical path
            nc.sync.dma_start(out=uidx_sb[:], in_=io["uidx"].ap())
            nc.sync.dma_start(out=midx_sb[:], in_=io["midx"].ap())
            if _gather and _nlayers > 2:
                # ALL gather preps, emitted at end of L1: descriptor
                # generation (2 x nch x 8.6us, serial on GpSimd Q7) fills
                # GpSimd idle time across L2/L3.  Table columns written
                # after this point go through the alias tensors; the
                # triggers carry manual deps on those writes.
                for ch in range(nch):
                    mg = gmp.tile([128, SC, TE], BF16, tag=f"mg{ch}")
                    _dgather(mg, mtab_d.ap(), midx_sb, ch, prep=True, q=1)
                    mgs.append(mg)
                for ch in range(nch):
                    ug = gup.tile([128, SC, TE], BF16, tag=f"ug{ch}")
                    _dgather(ug, utab_d.ap(), uidx_sb, ch, prep=True, q=2)
                    ugs.append(ug)
        if _gather and l == 2:
            # fire the prepped u-row gathers once utab cols 2,3 have landed
            utrig = nc.gpsimd.trigger_dma(count=None, queue_num=2)
            udeps = InstructionNameOrderedSet()
            for dma in ucol_dmas:
                udeps.add(dma.ins.name)
            utrig.ins.add_sync_dependencies_from(udeps)
            # The per-chunk sems (exact, not cumulative) gate each chunk's
            # consume via a relay that requires BOTH its gathers: two
            # GpSimd waits bump grelay[ch]; the consume waits grelay[ch]>=2.
            # no_sync deps pin the waits after the triggers in the GpSimd
            # stream (a hoisted wait before the trigger would deadlock).
            tdeps = InstructionNameOrderedSet()
            tdeps.add(mtrig[0].ins.name)
            tdeps.add(utrig.ins.name)
            for ch in range(nch):
                w1 = nc.gpsimd.wait_ge(gsem_m[ch], 16).then_inc(grelay[ch], 1)
                w1.ins.add_nosync_dependencies_from(tdeps)
                w2 = nc.gpsimd.wait_ge(gsem_u[ch], 16).then_inc(grelay[ch], 1)
                w2.ins.add_nosync_dependencies_from(tdeps)
            # consume: per-chunk product (in place, into the m tile)+reduce
            for ch in range(nch):
                mult = nc.vector.tensor_tensor(
                    mgs[ch][:], ugs[ch][:], mgs[ch][:], ALU.mult
                )
                mult._wait_ge(grelay[ch], 2)
                nc.vector.tensor_reduce(
                    res_sb[:, ch * SC : (ch + 1) * SC], mgs[ch][:],
                    axis=AXIS.X, op=ALU.add,
                )

    if not _gather:
        nc.any.memset(res_sb[:], 0.0)

    nc.sync.dma_start(out=io["res"].ap(), in_=res_sb[:])

    for p in reversed(ctxs):
        p.__exit__(None, None, None)


def _build(nch):
    ndev = 1 if os.environ.get("KSIM") else NCORES
    nc = bacc.Bacc("TRN2", num_devices=ndev, debug=False, num_swdge_queues=3)
    io = {}
    io["auT"] = nc.dram_tensor("auT", [NM, UP], F8, kind="ExternalInput")
    io["amT"] = nc.dram_tensor("amT", [NU, MP], F8, kind="ExternalInput")
    io["ustat0"] = nc.dram_tensor("ustat0", [128, UKT, E], F8, kind="ExternalInput")
    io["mstat0"] = nc.dram_tensor("mstat0", [128, MKT, E], F8, kind="ExternalInput")
    io["ueT"] = nc.dram_tensor("ueT", [E, UP], F32, kind="ExternalInput")
    io["meT"] = nc.dram_tensor("meT", [E, MP], F32, kind="ExternalInput")
    io["u0tab"] = nc.dram_tensor("u0tab", [UP, TE], BF16, kind="ExternalInput")
    io["m0tab"] = nc.dram_tensor("m0tab", [MP8, TE], BF16, kind="ExternalInput")
    io["wut"] = nc.dram_tensor("wut", [L, E, E], F32R, kind="ExternalInput")
    io["wmt"] = nc.dram_tensor("wmt", [L, E, E], F32R, kind="ExternalInput")
    io["bu2"] = nc.dram_tensor("bu2", [L, E, 1], F32, kind="ExternalInput")
    io["bm2"] = nc.dram_tensor("bm2", [L, E, 1], F32, kind="ExternalInput")
    io["wo4"] = nc.dram_tensor("wo4", [4, E, 1], F32, kind="ExternalInput")
    io["uidx"] = nc.dram_tensor("uidx", [128, nch * NW], I16, kind="ExternalInput")
    io["midx"] = nc.dram_tensor("midx", [128, nch * NW], I16, kind="ExternalInput")
    io["res"] = nc.dram_tensor("res", [128, nch * SC], F32, kind="ExternalOutput")

    with tile.TileContext(nc) as tc:
        _emit(nc, tc, io, nch)
    nc.compile()
    return nc


_cache = threading.local()


def _get_nc(nch):
    key = ("nc", nch)
    nc = getattr(_cache, "store", {}).get(key)
    if nc is None:
        nc = _build(nch)
        if not hasattr(_cache, "store"):
            _cache.store = {}
        _cache.store[key] = nc
    return nc


def _wrap_idx(arr, nch):
    """[CAP] int16 -> [128, nch*NW] wrapped layout for dma_gather."""
    w = arr.reshape(nch, NW, 16).transpose(2, 0, 1)   # [16, nch, NW]
    w = np.tile(w, (8, 1, 1)).reshape(128, nch * NW)
    return np.ascontiguousarray(w)


def _prep_in_maps(user_adj, movie_adj, user_emb, movie_emb, Wu, bu, Wm, bm,
                  Wo, bo, user_id, movie_id):
    user_adj = np.asarray(user_adj, np.float32)
    movie_adj = np.asarray(movie_adj, np.float32)
    user_emb = np.asarray(user_emb, np.float32)
    movie_emb = np.asarray(movie_emb, np.float32)
    Wu, bu = np.asarray(Wu, np.float32), np.asarray(bu, np.float32)
    Wm, bm = np.asarray(Wm, np.float32), np.asarray(bm, np.float32)
    Wo, bo = np.asarray(Wo, np.float32), np.asarray(bo, np.float32)
    user_id = np.asarray(user_id, np.int32)
    movie_id = np.asarray(movie_id, np.int32)

    wo = Wo[0]                                            # [(L+1)*E]
    wut = np.ascontiguousarray(Wu.transpose(0, 2, 1))
    wmt = np.ascontiguousarray(Wm.transpose(0, 2, 1))
    bu2 = np.ascontiguousarray((2.0 * bu).reshape(L, E, 1))
    bm2 = np.ascontiguousarray((2.0 * bm).reshape(L, E, 1))
    wo4 = np.ascontiguousarray(wo.reshape(4, E, 1))

    # stationaries: full embeddings, fp8 e4m3, [128, kt, 64] (zero-padded)
    uu = user_emb.astype(NPF8)                            # 16000 = 125*128
    ustat0 = np.ascontiguousarray(uu.reshape(UKT, 128, E).transpose(1, 0, 2))
    mm = np.zeros((MKT * 128, E), NPF8)                   # pad 8000 -> 8064
    mm[:NM] = movie_emb.astype(NPF8)
    mstat0 = np.ascontiguousarray(mm.reshape(MKT, 128, E).transpose(1, 0, 2))

    # movie table init: column 0 = natural movie embedding, rest zero
    m0tab = np.zeros((MP8, TE), NPBF16)
    m0tab[:NM, :E] = movie_emb.astype(NPBF16)

    # bucket pairs by uid owner
    own = user_id // UPC
    order = np.argsort(own, kind="stable")
    counts = np.bincount(own, minlength=NCORES)
    nch = max(1, -(-int(counts.max()) // CHUNK))
    cap = nch * CHUNK
    starts = np.zeros(NCORES + 1, np.int64)
    np.cumsum(counts, out=starts[1:])

    in_maps = []
    metas = []
    for c in range(NCORES):
        idx_c = order[starts[c] : starts[c + 1]]
        n_c = len(idx_c)
        uid_re = np.zeros(cap, np.int16)
        mid_c = np.zeros(cap, np.int16)
        uid_re[:n_c] = (user_id[idx_c] - c * UPC).astype(np.int16)
        mid_c[:n_c] = movie_id[idx_c].astype(np.int16)

        auT = np.zeros((NM, UP), NPF8)
        auT[:, :UPC] = (
            user_adj[c * UPC : (c + 1) * UPC].T * np.float32(ASCALE)
        ).astype(NPF8)
        amT = np.zeros((NU, MP), NPF8)
        amT[:, :MPC] = (
            movie_adj[c * MPC : (c + 1) * MPC].T * np.float32(ASCALE)
        ).astype(NPF8)

        ue_sl = np.zeros((UP, E), np.float32)
        ue_sl[:UPC] = user_emb[c * UPC : (c + 1) * UPC]
        me_sl = np.zeros((MP, E), np.float32)
        me_sl[:MPC] = movie_emb[c * MPC : (c + 1) * MPC]

        # U table init: column 0 = u0 * wo0 (local users, bf16), rest zero
        u0tab = np.zeros((UP, TE), NPBF16)
        u0tab[:, :E] = (ue_sl * wo[:E][None, :]).astype(NPBF16)

        in_maps.append({
            "auT": auT,
            "amT": amT,
            "ustat0": ustat0,
            "mstat0": mstat0,
            "ueT": np.ascontiguousarray(ue_sl.T) * np.float32(ASCALE),
            "meT": np.ascontiguousarray(me_sl.T) * np.float32(ASCALE),
            "u0tab": u0tab,
            "m0tab": m0tab,
            "wut": wut,
            "wmt": wmt,
            "bu2": bu2,
            "bm2": bm2,
            "wo4": wo4,
            "uidx": _wrap_idx(uid_re, nch),
            "midx": _wrap_idx(mid_c, nch),
        })
        metas.append((idx_c, n_c))

    return in_maps, metas, float(bo[0]), nch


def _postprocess(results, metas, bo0, nch):
    cap = nch * CHUNK
    out = np.zeros(B, np.float32)
    for c in range(NCORES):
        idx_c, n_c = metas[c]
        r = results[c]["res"]                             # [128, nch*SC]
        vals = r.reshape(128, nch, SC).transpose(1, 2, 0).reshape(cap)
        out[idx_c] = vals[:n_c]
    return out + np.float32(bo0)


def kernel(user_adj, movie_adj, user_emb, movie_emb, Wu, bu, Wm, bm, Wo, bo,
           user_id, movie_id):
    in_maps, metas, bo0, nch = _prep_in_maps(
        user_adj, movie_adj, user_emb, movie_emb, Wu, bu, Wm, bm, Wo, bo,
        user_id, movie_id,
    )
    nc = _get_nc(nch)
    res = run_bass_kernel_spmd(nc, in_maps, core_ids=list(range(NCORES)))
    return _postprocess(res.results, metas, bo0, nch)
